# revision 44
# baseline (speedup 1.0000x reference)
"""Trainium2 Bass kernel for NaiveRNN.

Reference computation:
    xi = x @ W_i2h + b_i2h                      # [B, L, D_h]
    h_{t+1} = tanh(xi_t + h_t @ W_h2h + b_h2h)  # L sequential steps
    out = h_L @ W_out + b_out                   # [B, D_out]

Sharding: data-parallel over batch B=128 across 8 cores (16 rows each).
Weights replicated. No cross-core communication.

Per-core kernel design (all bf16 on the PE, f32 PSUM accumulation):

  Phase 1: xi' = x @ W_i2h + (b_i2h + b_h2h), written to DRAM scratch
      [L, B_loc, D_h] bf16. x arrives pre-transposed from the host
      (xT[r*D_in+din, l] = x[128r+l, din]) so no on-device transposes
      are needed; xT tiles cast-load bf16 on the gpsimd (SWDGE) ring,
      xi writes go on the sync (HWDGE) ring -- each DMA FIFO carries one
      traffic class, so prefetch never head-of-line blocks and the PE
      stays HAM-warm. Bias injected via a zero-padded K=128 ones matmul
      so every matmul keeps the (128,128) array mode (no drains).

  Phase 2: 512 recurrence steps, two independent batch-8 recurrences
      ("groups") interleaved back-to-back so each group's tanh/transpose
      tail hides under the other group's matmul pass. The batch uses only
      8 of the PE's 128 stationary columns, so the array runs in 128x32
      column-tiling mode: 4 concurrent matmuls per k-round, each
      streaming a different quarter of W_h2h's columns (N=256) -> ~4x
      less PE streaming time per step than a single-tile schedule.

      Column->tile assignment is interleaved at 32-column granularity
      (tile j takes dh columns with (dh//32)%4 == j, realized purely by
      strided access patterns on the W/xi streams: tiles laid out as
      [p, k, cc, j, w] are bit-identical to the plain layout). With this
      assignment the tanh output h_new[32j+b, 32cc+w] = h[b, 128cc+32j+w]
      turns into the next step's stationary operand hT[p, 32k+v] =
      h[v, 128k+p] via an in-place 32x32 block transpose -- which is
      exactly what the DVE's stream-transpose instruction does.
      Per-group per-step recurrent path:
        PE: 4 xi-inject matmuls (zero-padded identity, K=128, hoisted
            into the other group's round stream to hide their LDWEIGHTS)
            + 8 rounds x 4 col-tiled matmuls (N=256)   -> PSUM [128,256]
        ACT: tanh PSUM -> SBUF bf16, in pieces of (2,3,3) 32-col chunks
        DVE: stream-transpose per piece -> hT piece tiles, so early
            chunks unblock the next pass's early k-rounds
      No DMA and no PE-transpose on the critical path (the baseline lost
      ~5us/step to DMA-transpose latency + HAM cold-clock oscillation).

  Phase 3: out = h_L @ W_out + b_out (bf16 matmuls, f32 out), both
      groups' head matmuls into 32-aligned PSUM row groups.
"""

import numpy as np

B, L, D_IN, D_H, D_OUT = 128, 512, 512, 1024, 512
NCORES = 8
BL = B // NCORES            # 16 local batch rows
KI = D_IN // 128            # 4 k-chunks for input proj
KH = D_H // 128             # 8 k-chunks for recurrence
ROW_TILES = (BL * L) // 128  # 64 row tiles in phase 1
LW = L // 128               # l-windows per batch row (4)
NT = 4                      # column tiles in phase 2


def build_nc(l_steps=L):
    import concourse.bass as bass
    import concourse.mybir as mybir
    from concourse import bacc
    from concourse.tile import TileContext
    from concourse.masks import make_identity

    dt = mybir.dt
    f32, bf16 = dt.float32, dt.bfloat16
    AF = mybir.ActivationFunctionType

    nc = bacc.Bacc(
        "TRN2", target_bir_lowering=False, debug=False, num_devices=NCORES
    )
    # x arrives pre-transposed per 128-row tile (host-side prep):
    # xT[r*D_IN + din, l] = x[128*r + l, din]. This kills the on-device
    # transpose entirely (DMA-transposes serialize against other DMA
    # traffic in the scheduler and were gating phase 1).
    xT_dram = nc.dram_tensor(
        "xT", [ROW_TILES * D_IN, 128], f32, kind="ExternalInput"
    )
    W_i2h = nc.dram_tensor("W_i2h", [D_IN, D_H], f32, kind="ExternalInput")
    b_i2h = nc.dram_tensor("b_i2h", [D_H], f32, kind="ExternalInput")
    W_h2h = nc.dram_tensor("W_h2h", [D_H, D_H], f32, kind="ExternalInput")
    b_h2h = nc.dram_tensor("b_h2h", [D_H], f32, kind="ExternalInput")
    W_out = nc.dram_tensor("W_out", [D_H, D_OUT], f32, kind="ExternalInput")
    b_out = nc.dram_tensor("b_out", [D_OUT], f32, kind="ExternalInput")
    out = nc.dram_tensor("out", [BL, D_OUT], f32, kind="ExternalOutput")
    xi_dram = nc.dram_tensor("xi_scratch", [L, BL, D_H], bf16, kind="Internal")

    with TileContext(nc) as tc:
        with tc.tile_pool(name="const", bufs=1) as cpool:
            # Persistent weights/constants in SBUF; SWDGE casts f32->bf16.
            # whh laid out [p, k, cc, j, w] == plain [p, k, n] bit-for-bit;
            # slicing [:, k, :, j, :] yields the interleaved N=256 stream
            # for column-tile j.
            whh = cpool.tile([128, KH, KH, NT, 32], bf16, tag="whh")
            wi2h = cpool.tile([128, KI, D_H], bf16, tag="wi2h")
            wout = cpool.tile([128, KH, D_OUT], bf16, tag="wout")
            # wi2h first: phase 1 starts as soon as it lands; whh/wout
            # (phase 2/3 only) are deferred into the phase-1 loop.
            nc.gpsimd.dma_start(
                wi2h[:], W_i2h.ap().rearrange("(k p) n -> p k n", p=128)
            )
            # K=128 zero-padded identity: rows 0-15 = I16, rest 0.
            ipad = cpool.tile([128, BL], bf16, tag="ipad")
            nc.gpsimd.memset(ipad[:], 0.0)
            make_identity(nc, ipad[:BL, :BL], nomemset=True)
            # row 32 = ones: pairs with the combined bias stored in row 32
            # of the xi ring buffers so the inject adds b_i2h+b_h2h free.
            nc.gpsimd.memset(ipad[32:33, :], 1.0)
            # K=128 zero-padded ones row (for bias injection matmuls).
            ones_pad = cpool.tile([128, 128], bf16, tag="ones_pad")
            nc.gpsimd.memset(ones_pad[:], 0.0)
            nc.gpsimd.memset(ones_pad[:1, :], 1.0)
            # combined bias (b_i2h + b_h2h), zero-padded to K=128 rows.
            bi = cpool.tile([1, D_H], f32, tag="bi")
            nc.sync.dma_start(bi[:], b_i2h.ap().unsqueeze(0))
            bh = cpool.tile([1, D_H], f32, tag="bh")
            nc.sync.dma_start(bh[:], b_h2h.ap().unsqueeze(0))
            bcomb = cpool.tile([128, D_H], bf16, tag="bcomb")
            nc.gpsimd.memset(bcomb[:], 0.0)
            nc.vector.tensor_add(bcomb[:1, :], bi[:], bh[:])
            # phase-3 bias (K=1 matmul; single mode switch is fine there)
            bo_f = cpool.tile([1, D_OUT], f32, tag="bo_f")
            nc.sync.dma_start(bo_f[:], b_out.ap().unsqueeze(0))
            bo = cpool.tile([1, D_OUT], bf16, tag="bo")
            nc.vector.tensor_copy(bo[:], bo_f[:])
            ones_row = cpool.tile([1, 128], bf16, tag="ones")
            nc.gpsimd.memset(ones_row[:], 1.0)

            # ---------------- Phase 1: xi' = x @ W_i2h + bcomb ----------------
            # Loads alone on the gpsimd ring (free-flowing prefetch), xi
            # writes alone on the sync ring: no DMA head-of-line blocking.
            with (
                tc.tile_pool(name="p1", bufs=6) as p1pool,
                tc.tile_pool(name="p1ps", bufs=3, space="PSUM") as p1ps,
            ):
                # lw-major order: all 16 batch rows' l-window lw complete
                # before lw+1, so phase-2 steps [128*lw, 128*(lw+1)) become
                # eligible as early as possible.
                for idx in range(ROW_TILES):
                    lw, b_idx = idx // BL, idx % BL
                    r = b_idx * LW + lw
                    if idx == 4:
                        nc.gpsimd.dma_start(
                            whh[:].rearrange("p k a j w -> p k (a j w)"),
                            W_h2h.ap().rearrange("(k p) n -> p k n", p=128),
                        )
                        nc.gpsimd.dma_start(
                            wout[:],
                            W_out.ap().rearrange("(k p) n -> p k n", p=128),
                        )
                    xT = p1pool.tile([128, KI, 128], bf16, tag="xT")
                    nc.gpsimd.dma_start(
                        xT[:],
                        xT_dram.ap()[D_IN * r : D_IN * (r + 1), :].rearrange(
                            "(k p) l -> p k l", p=128
                        ),
                    )
                    xi_sb = p1pool.tile([128, D_H], bf16, tag="xi_sb")
                    for h in range(2):
                        ns = slice(512 * h, 512 * h + 512)
                        zp = p1ps.tile([128, 512], f32, tag="zp1")
                        for k in range(KI):
                            nc.tensor.matmul(
                                zp[:],
                                xT[:, k, :],
                                wi2h[:, k, ns],
                                start=(k == 0),
                                stop=(k == KI - 1),
                            )
                        nc.scalar.activation(xi_sb[:, ns], zp[:], AF.Copy)
                    nc.sync.dma_start(
                        xi_dram[128 * lw : 128 * lw + 128, b_idx, :], xi_sb[:]
                    )

            # ---------------- Phase 2: recurrence ----------------
            with (
                tc.tile_pool(name="p2h", bufs=2) as hpool,
                tc.tile_pool(name="p2ps", bufs=2, space="PSUM") as p2ps,
            ):
                # xi ring: [p, cc, j, w] == plain [p, n]; rows 16-127 are
                # zeroed once (finite junk x zero ipad rows = exact 0).
                # layout shim: phase-2 performance is sensitive to the SBUF
                # placement of the recurrence tiles (measured ~0.5us/step
                # swing); this pad selects the faster alignment class.
                lay_pad = cpool.tile([128, 1024], bf16, tag="lay_pad")
                xi_bufs = [
                    cpool.tile(
                        [128, KH, NT, 32], bf16, tag=f"xi{i}", name=f"xi{i}"
                    )
                    for i in range(3)
                ]
                for t_ in xi_bufs:
                    nc.gpsimd.memset(t_[:], 0.0)
                    # persistent combined bias in row 32 (ipad row 32 ones)
                    nc.vector.tensor_add(
                        t_[32:33].rearrange("p a j w -> p (a j w)"),
                        bi[:],
                        bh[:],
                    )
                # Two independent batch-8 recurrences ("groups") interleaved
                # on the PE: group B's matmul rounds execute during group A's
                # tanh+transpose tail and vice versa, so the PE never waits
                # on the serial ACT/DVE chain.
                GB = BL // 2
                # tanh+transpose piece boundaries (in 32-col chunks): the
                # first piece is small so the chunk-0 transpose lands just
                # before the next pass consumes it; later pieces track the
                # round-consumption schedule (one chunk per ~109ns slot).
                # One hT tile per piece gives Tile piece-granular deps.
                PIECES = [(0, 2), (2, 5), (5, 8)]
                PIECE_OF = {
                    k: pi
                    for pi, (c0, c1) in enumerate(PIECES)
                    for k in range(c0, c1)
                }
                # per-group transposed state, ping-pong, one tile per piece
                hT = [
                    [
                        [
                            cpool.tile(
                                [128, c1 - c0, 32],
                                bf16,
                                tag=f"hT{g_}{p_}{pi}",
                                name=f"hT{g_}{p_}{pi}",
                            )
                            for pi, (c0, c1) in enumerate(PIECES)
                        ]
                        for p_ in range(2)
                    ]
                    for g_ in range(2)
                ]
                for g_ in range(2):
                    for pi in range(len(PIECES)):
                        nc.gpsimd.memset(hT[g_][0][pi][:], 0.0)
                # PSUM ping-pong tiles; zero once so the never-written junk
                # rows of each 32-row group stay finite for tanh.
                zeros_sb = cpool.tile([128, 256], f32, tag="zeros_sb")
                nc.gpsimd.memset(zeros_sb[:], 0.0)
                zps = [
                    [
                        p2ps.tile(
                            [128, 256], f32, tag=f"zp2_{g_}", name=f"zp2_{g_}{i}"
                        )
                        for i in range(2)
                    ]
                    for g_ in range(2)
                ]
                for zg in zps:
                    for z_ in zg:
                        nc.vector.tensor_copy(z_[:], zeros_sb[:])

                def emit_xi_load(t):
                    xi_t = xi_bufs[t % 3]
                    nc.gpsimd.dma_start(
                        xi_t[:BL].rearrange("p a j w -> p (a j w)"),
                        xi_dram[t, :, :],
                    )

                def emit_inject(g, t):
                    # xi inject: 4 col-tiled K=128 identity matmuls.
                    # ipad[:, 8g:8g+8] selects xi rows 8g..8g+8. Hoisted
                    # into the OTHER group's round stream so the exposed
                    # ~100ns LDWEIGHTS hides under running matmuls.
                    zp = zps[g][t % 2]
                    xi_t = xi_bufs[t % 3]
                    for j in range(NT):
                        nc.tensor.matmul(
                            zp[32 * j : 32 * j + GB, :],
                            ipad[:, GB * g : GB * g + GB],
                            xi_t[:, :, j, :],
                            start=True,
                            stop=False,
                            tile_position=(0, 32 * j),
                        )

                def emit_rounds(g, t, ks):
                    zp = zps[g][t % 2]
                    hT_cur = hT[g][t % 2]
                    for k in ks:
                        pi = PIECE_OF[k]
                        lhsT = hT_cur[pi][:, k - PIECES[pi][0], :GB]
                        for j in range(NT):
                            nc.tensor.matmul(
                                zp[32 * j : 32 * j + GB, :],
                                lhsT,
                                whh[:, k, :, j, :],
                                start=False,
                                stop=(k == KH - 1),
                                tile_position=(0, 32 * j),
                            )

                def emit_tail(g, t):
                    # tanh + in-place 32x32 block transpose, piecewise so
                    # early chunks unblock the next pass's early k-rounds.
                    zp = zps[g][t % 2]
                    hT_nxt = hT[g][(t + 1) % 2]
                    h_new = hpool.tile(
                        [128, KH, 32], bf16, tag=f"h_new{g}",
                        name=f"h_new{g}_{t}",
                    )
                    for pi, (c0, c1) in enumerate(PIECES):
                        hs = slice(c0, c1)
                        nc.scalar.activation(
                            h_new[:, hs, :].rearrange("p a w -> p (a w)"),
                            zp[:, 32 * c0 : 32 * c1],
                            AF.Tanh,
                        )
                        nc.vector.transpose(
                            hT_nxt[pi][:].rearrange("p a w -> p (a w)"),
                            h_new[:, hs, :].rearrange("p a w -> p (a w)"),
                        )

                emit_xi_load(0)
                emit_xi_load(1)
                emit_inject(0, 0)
                for t in range(l_steps):
                    if t + 2 < l_steps:
                        emit_xi_load(t + 2)
                    emit_rounds(0, t, range(KH - 1))
                    emit_inject(1, t)
                    emit_rounds(0, t, [KH - 1])
                    emit_tail(0, t)
                    emit_rounds(1, t, range(KH - 1))
                    if t + 1 < l_steps:
                        emit_inject(0, t + 1)
                    emit_rounds(1, t, [KH - 1])
                    emit_tail(1, t)

                # ---------------- Phase 3: head ----------------
                # group A -> psum rows 0-7, group B -> rows 32-39 (col-tile
                # base partitions must be 32-aligned), then recombine.
                zp3 = p2ps.tile([128, D_OUT], f32, tag="zp3")
                for g in range(2):
                    base = 32 * g
                    hT_fin = hT[g][l_steps % 2]
                    nc.tensor.matmul(
                        zp3[base : base + GB, :],
                        ones_row[:, :GB],
                        bo[:],
                        start=True,
                        stop=False,
                        tile_position=(0, base),
                    )
                    for k in range(KH):
                        pi = PIECE_OF[k]
                        nc.tensor.matmul(
                            zp3[base : base + GB, :],
                            hT_fin[pi][:, k - PIECES[pi][0], :GB],
                            wout[:, k, :],
                            start=False,
                            stop=(k == KH - 1),
                            tile_position=(0, base),
                        )
                out_sb = cpool.tile([128, D_OUT], f32, tag="out_sb")
                nc.vector.tensor_copy(out_sb[:GB], zp3[:GB])
                nc.vector.tensor_copy(
                    out_sb[32 : 32 + GB], zp3[32 : 32 + GB]
                )
                nc.sync.dma_start(out.ap()[:GB], out_sb[:GB])
                nc.sync.dma_start(out.ap()[GB:BL], out_sb[32 : 32 + GB])

    nc.compile()
    return nc


_CACHE = {}


def _get_nc(l_steps=L):
    if l_steps not in _CACHE:
        _CACHE[l_steps] = build_nc(l_steps)
    return _CACHE[l_steps]


def run(inputs, l_steps=L, trace=False, tmpdir=None):
    from concourse.bass_utils import run_bass_kernel_spmd

    nc = _get_nc(l_steps)
    x = np.asarray(inputs["x"], np.float32).reshape(B, L, D_IN)
    shared = {
        k: np.ascontiguousarray(np.asarray(inputs[k], np.float32))
        for k in ("W_i2h", "b_i2h", "W_h2h", "b_h2h", "W_out", "b_out")
    }
    in_maps = []
    for c in range(NCORES):
        m = dict(shared)
        xl = x[c * BL : (c + 1) * BL].reshape(ROW_TILES, 128, D_IN)
        m["xT"] = np.ascontiguousarray(xl.transpose(0, 2, 1)).reshape(
            ROW_TILES * D_IN, 128
        )
        in_maps.append(m)
    res = run_bass_kernel_spmd(
        nc,
        in_maps,
        core_ids=list(range(NCORES)),
        trace=trace,
        tmpdir=tmpdir,
    )
    out = np.concatenate([r["out"] for r in res.results], axis=0)
    return out, res


def kernel(**inputs) -> np.ndarray:
    out, _ = run(inputs)
    return out
